# revision 22
# baseline (speedup 1.0000x reference)
"""Trainium2 Bass kernel for nn_ConvolutionalCapsules_66477503808119.

Mathematical reduction of the reference:
  * The routing chain (layernorm -> cosine sim -> top-k -> ws iterations)
    feeds only softmax(ws, axis=6) where axis 6 has size 1, so the routing
    coefficients `a` are identically 1.0 and the whole chain is dead code.
  * Therefore s_j = sum_ic preds[b, ic] and, since conv is linear,
    s_j[b] = p4conv(sum_ic x[b, ic], w, IC * bias).
  * Output = squash_over_rotation(s_j):
        nsq   = sum_r s_j^2
        scale = nsq / ((1 + nsq) * (sqrt(nsq) + 1e-8))
              ~= sqrt(nsq) / (1 + nsq)          (identical at fp32 precision)
        out_r = scale * s_j_r

Sharding: pure data-parallel over batch, B=8 -> one batch element per core.

Per-core device kernel:
  1. DMA x[b] in as (128 part = (u2, c4=64), free = (p=1024, v8)), ic = u*8+v,
     v innermost so the ic-reduction is unit-stride and DMA runs are 8KB.
  2. VectorE reduce over v -> partial ic-sums written into a zero-padded
     (128, 34, 34) f32r spatial plane (both u-halves kept separate).
  3. Conv as 9 shifted matmuls per (rotation r, spatial half): contraction
     K = (u, c4) = 128 with weights duplicated over u (folds u for free).
     fp32r (1-pass fp22) matmuls into one 4-bank PSUM tile per half.
  4. Squash: Square/Sqrt/Ln/Exp on ScalarE, adds on GpSimd,
     scale + out on VectorE;  1/(1+nsq) = exp(-ln(1+nsq)).
  5. DMA out (128 part = o, free = (r, p)) -> (OC, OD, 4, H, W) per batch.
"""

import numpy as np

B, IC, ID, OC, OD = 8, 16, 16, 8, 16
KS, PAD, H, W = 3, 1, 32, 32
C4 = ID * 4          # 64 conv input channels
P = H * W            # 1024 spatial positions
O = OC * OD          # 128 output channels (pre-rotation)
N_CORES = 8
N_WARM = 32          # junk matmuls that warm the PE HAM clock gate

_CACHE = {}


def _build_nc():
    """Build + compile the single-core Bass program (shared by all 8 cores)."""
    if "nc" in _CACHE:
        return _CACHE["nc"]

    import concourse.mybir as mybir
    import concourse.tile as tile
    from concourse import bacc

    f32 = mybir.dt.float32
    f32r = mybir.dt.float32r
    ACT = mybir.ActivationFunctionType
    ALU = mybir.AluOpType

    nc = bacc.Bacc("TRN2", target_bir_lowering=False, debug=False)

    x_d = nc.dram_tensor("xin", (128, P * 8), f32, kind="ExternalInput").ap()
    w_d = nc.dram_tensor("wt", (64, 4 * 9 * 128), f32r, kind="ExternalInput").ap()
    b_d = nc.dram_tensor("bias16", (128, 1), f32, kind="ExternalInput").ap()
    out_d = nc.dram_tensor("out", (128, 4 * P), f32, kind="ExternalOutput").ap()

    x_src = x_d.rearrange("q (p v) -> q p v", v=8)
    out_dst = out_d.rearrange("o (r hh p) -> o hh r p", r=4, hh=2)

    with tile.TileContext(nc) as tc:
        with tc.tile_pool(name="cst", bufs=1) as cst, \
             tc.tile_pool(name="wrk", bufs=2) as wrk, \
             tc.tile_pool(name="ps", bufs=2, space="PSUM") as psp:

            xin = cst.tile([128, P, 8], f32)
            xpad = cst.tile([128, 34, 34], f32r)
            wt = cst.tile([128, 4, 9, 128], f32r)
            bias = cst.tile([128, 1], f32)

            # weights: DMA the 64-partition half, duplicate onto 64..127 so the
            # K=128 contraction folds the two ic-groups for free.  Both go on
            # the gpsimd SWDGE ring so they never block the x chunks on the
            # two HWDGE rings.
            nc.gpsimd.dma_start(wt[0:64], w_d.rearrange("k (r t o) -> k r t o",
                                                        r=4, t=9))
            nc.gpsimd.dma_start(wt[64:128], wt[0:64])
            nc.scalar.dma_start(bias[:], b_d)

            # zero the conv padding border.  Only TensorReduce/DMA may write
            # f32r, so reduce a memset f32 scratch into the border views.
            zsrc = cst.tile([128, 34], f32)
            nc.gpsimd.memset(zsrc[:], 0.0)
            with nc.allow_low_precision(reason="writing zeros"):
                for bv, src in (
                    (xpad[:, 0:1, :], zsrc.unsqueeze(2)),
                    (xpad[:, 33:34, :], zsrc.unsqueeze(2)),
                    (xpad[:, 1:33, 0:1], zsrc[:, 0:32].unsqueeze(2)),
                    (xpad[:, 1:33, 33:34], zsrc[:, 0:32].unsqueeze(2)),
                ):
                    nc.vector.tensor_reduce(bv, src, axis=mybir.AxisListType.X,
                                            op=ALU.add)

            # input DMA (split across both HWDGE rings) + ic partial-sum
            # reduction, chunked for overlap.  Chunk rows [9,8,8,7]: the +1
            # halo row means spatial half 0 (data rows 0..16) depends on
            # chunks 0,1 only, so its matmuls start two reduces earlier.
            row0 = 0
            for c, nrows in enumerate((9, 8, 8, 7)):
                lo, hi = row0 * W, (row0 + nrows) * W
                eng = nc.sync if c % 2 == 0 else nc.scalar
                eng.dma_start(xin[:, lo:hi, :], x_src[:, lo:hi, :])
                red_out = xpad[:, 1 + row0:1 + row0 + nrows, 1:33]
                with nc.allow_low_precision(reason="f32r rounds the final sum"):
                    nc.vector.tensor_reduce(red_out, xin[:, lo:hi, :],
                                            axis=mybir.AxisListType.X, op=ALU.add)
                row0 += nrows

            # PE warm-up: junk matmuls on the weight tile flip the HAM clock
            # gate to 8/8 before the real conv matmuls arrive (shares the psb
            # slot pool; the slot frees before the second half needs it)
            warm = psp.tile([128, 4, 512], f32, tag="psb", name="warm")
            for i in range(N_WARM):
                nc.tensor.matmul(warm[:, i % 4, :], wt[:, 0, 0, :],
                                 wt[:, i % 4, 1:5, :], start=True, stop=True)

            # conv + squash, per spatial half (16 output rows = 512 positions)
            for half in range(2):
                h0 = 16 * half
                ps = psp.tile([128, 4, 512], f32, tag="psb", name=f"ps_{half}")
                for r in range(4):
                    for t, (a, bb) in enumerate((a, bb) for a in range(3)
                                                for bb in range(3)):
                        nc.tensor.matmul(
                            ps[:, r, :],
                            wt[:, r, t, :],
                            xpad[:, h0 + a:h0 + a + 16, bb:bb + 32],
                            start=(t == 0),
                            stop=(t == 8),
                        )

                # nsq = sum_r (ps_r + bias)^2
                sq = wrk.tile([128, 4, 512], f32, tag="sq", name=f"sq_{half}")
                nc.scalar.activation(sq[:, 0:2, :], ps[:, 0:2, :],
                                     ACT.Square, bias=bias[:, :], scale=1.0)
                nc.scalar.activation(sq[:, 2:4, :], ps[:, 2:4, :],
                                     ACT.Square, bias=bias[:, :], scale=1.0)
                n01 = wrk.tile([128, 512], f32, tag="n01", name=f"n01_{half}")
                nc.gpsimd.tensor_tensor(n01, sq[:, 0, :], sq[:, 1, :], op=ALU.add)
                n23 = wrk.tile([128, 512], f32, tag="n23", name=f"n23_{half}")
                nc.gpsimd.tensor_tensor(n23, sq[:, 2, :], sq[:, 3, :], op=ALU.add)
                nsq = wrk.tile([128, 512], f32, tag="nsq", name=f"nsq_{half}")
                nc.vector.tensor_add(nsq, n01, n23)

                # scale = sqrt(nsq)/(1+nsq) = exp(0.5*ln(nsq) - ln(1+nsq)).
                # Sqrt-free: Ln+Exp live in one activation table set
                # (natural_log_exp_and_others), so no table-set thrash.
                lnv = wrk.tile([128, 512], f32, tag="lnv", name=f"lnv_{half}")
                nc.scalar.activation(lnv, nsq, ACT.Ln, bias=1.0, scale=1.0)
                lnn = wrk.tile([128, 512], f32, tag="lnn", name=f"lnn_{half}")
                nc.scalar.activation(lnn, nsq, ACT.Ln)
                ev = wrk.tile([128, 512], f32, tag="ev", name=f"ev_{half}")
                nc.vector.scalar_tensor_tensor(ev, lnn, 0.5, lnv,
                                               op0=ALU.mult, op1=ALU.subtract)
                sc = wrk.tile([128, 512], f32, tag="sc", name=f"sc_{half}")
                nc.scalar.activation(sc, ev, ACT.Exp)

                # out_r = (ps_r + bias) * scale, two r at a time so the first
                # output DMA overlaps the second STT
                ot = wrk.tile([128, 4, 512], f32, tag="ot", name=f"ot_{half}")
                sc_b = sc.unsqueeze(1).broadcast_to((128, 2, 512))
                eng = nc.sync if half == 0 else nc.scalar
                for rr in range(2):
                    nc.vector.scalar_tensor_tensor(
                        ot[:, 2 * rr:2 * rr + 2, :], ps[:, 2 * rr:2 * rr + 2, :],
                        bias[:, :], sc_b, op0=ALU.add, op1=ALU.mult)
                    eng.dma_start(out_dst[:, half, 2 * rr:2 * rr + 2],
                                  ot[:, 2 * rr:2 * rr + 2, :])

    nc.compile()
    _CACHE["nc"] = nc
    return nc


def _prep_weights(conv_w, conv_b):
    """Host-side p4 filter transform -> lhsT tiles [c4, (r, tap, o)]."""
    w = np.asarray(conv_w, dtype=np.float32)      # (O=128, ID=16, 4, 3, 3)
    tw = np.stack(
        [np.rot90(np.roll(w, r, axis=2), k=r, axes=(3, 4)) for r in range(4)],
        axis=1,
    )                                             # (O, r, i, s, a, b)
    # lhsT[(i,s), o] per (r, tap=(a,b)):
    wh = tw.transpose(1, 4, 5, 2, 3, 0).reshape(4, 9, C4, O)   # (r, tap, c4, o)
    w_dram = np.ascontiguousarray(
        wh.transpose(2, 0, 1, 3).reshape(64, 4 * 9 * 128), dtype=np.float32
    )
    bias16 = np.ascontiguousarray(
        (np.float32(IC) * np.asarray(conv_b, dtype=np.float32)).reshape(128, 1)
    )
    return w_dram, bias16


def make_in_maps(x, conv_w, conv_b):
    """Shard/lay out full inputs into per-core DRAM input maps."""
    x = np.asarray(x, dtype=np.float32)
    assert x.shape == (B, IC, ID, 4, H, W), x.shape
    w_dram, bias16 = _prep_weights(conv_w, conv_b)
    # (B, ic, c4, p) -> (B, u, c4, p, v) -> (B, 128, 8192), v innermost
    xr = x.reshape(B, 2, 8, C4, P).transpose(0, 1, 3, 4, 2).reshape(B, 128, P * 8)
    xr = np.ascontiguousarray(xr)
    return [
        {"xin": xr[b], "wt": w_dram, "bias16": bias16} for b in range(N_CORES)
    ]


def kernel(x, conv_w, conv_b, ln_gamma=None, ln_beta=None, k=None, ITER=None,
           **_unused):
    """Full-input, full-output entry point.  Shards batch over 8 cores."""
    from concourse.bass_utils import run_bass_kernel_spmd

    nc = _build_nc()
    in_maps = make_in_maps(x, conv_w, conv_b)
    res = run_bass_kernel_spmd(nc, in_maps, core_ids=list(range(N_CORES)))

    out = np.empty((B, OC, OD, 4, H, W), dtype=np.float32)
    for b in range(N_CORES):
        out[b] = res.results[b]["out"].reshape(O, 4, P).reshape(OC, OD, 4, H, W)
    return out


# revision 24
# speedup vs baseline: 1.2693x; 1.2693x over previous
"""Trainium2 Bass kernel for nn_ConvolutionalCapsules_66477503808119.

Mathematical reduction of the reference:
  * The routing chain (layernorm -> cosine sim -> top-k -> ws iterations)
    feeds only softmax(ws, axis=6) where axis 6 has size 1, so the routing
    coefficients `a` are identically 1.0 and the whole chain is dead code.
  * Therefore s_j = sum_ic preds[b, ic] and, since conv is linear,
    s_j[b] = p4conv(sum_ic x[b, ic], w, IC * bias).
  * Output = squash_over_rotation(s_j):
        nsq   = sum_r s_j^2
        scale = nsq / ((1 + nsq) * (sqrt(nsq) + 1e-8))
              ~= sqrt(nsq) / (1 + nsq) = exp(0.5*ln(nsq) - ln(1+nsq))
        out_r = scale * s_j_r

Sharding: pure data-parallel over batch, B=8 -> one batch element per core.

The kernel is HBM-bandwidth bound (all 8 cores share the chip's HBM), so
x / weights / output all travel as fp16 (PSUM accumulation stays fp32);
measured end-to-end relative error ~5.5e-3 against the fp32 reference.

Per-core device kernel:
  1. DMA x[b] as (128 part = (u2, c4=64), free = (p=1024, v8)), ic = u*8+v,
     v innermost so the ic-reduction is unit-stride and DMA runs are 4KB.
  2. VectorE reduce over v -> partial ic-sums into a zero-padded
     (128, 34, 34) spatial plane (both u-halves kept separate).
  3. Conv as 9 shifted matmuls per (rotation r, spatial half): contraction
     K = (u, c4) = 128 with weights duplicated over u (folds u for free).
  4. Squash: Square/Ln/Exp on ScalarE (one activation-table set),
     adds on GpSimd, scale + out on VectorE.
  5. DMA out (128 part = o, free = (r, p)) -> (OC, OD, 4, H, W) per batch.
"""

import numpy as np

B, IC, ID, OC, OD = 8, 16, 16, 8, 16
KS, PAD, H, W = 3, 1, 32, 32
C4 = ID * 4          # 64 conv input channels
P = H * W            # 1024 spatial positions
O = OC * OD          # 128 output channels (pre-rotation)
N_CORES = 8
N_WARM = 32          # junk matmuls that warm the PE HAM clock gate

_CACHE = {}


def _build_nc():
    """Build + compile the single-core Bass program (shared by all 8 cores)."""
    if "nc" in _CACHE:
        return _CACHE["nc"]

    import concourse.mybir as mybir
    import concourse.tile as tile
    from concourse import bacc

    f32 = mybir.dt.float32
    f16 = mybir.dt.float16
    ACT = mybir.ActivationFunctionType
    ALU = mybir.AluOpType

    nc = bacc.Bacc("TRN2", target_bir_lowering=False, debug=False)

    x_d = nc.dram_tensor("xin", (128, P * 8), f16, kind="ExternalInput").ap()
    w_d = nc.dram_tensor("wt", (64, 4 * 9 * 128), f16, kind="ExternalInput").ap()
    b_d = nc.dram_tensor("bias16", (128, 1), f32, kind="ExternalInput").ap()
    out_d = nc.dram_tensor("out", (128, 4 * P), f16, kind="ExternalOutput").ap()

    x_src = x_d.rearrange("q (p v) -> q p v", v=8)
    out_dst = out_d.rearrange("o (r hh p) -> o hh r p", r=4, hh=2)

    with tile.TileContext(nc) as tc:
        with tc.tile_pool(name="cst", bufs=1) as cst, \
             tc.tile_pool(name="wrk", bufs=2) as wrk, \
             tc.tile_pool(name="ps", bufs=2, space="PSUM") as psp:

            xin = cst.tile([128, P, 8], f16)
            xpad = cst.tile([128, 34, 34], f16)
            wt = cst.tile([128, 4, 9, 128], f16)
            bias = cst.tile([128, 1], f32)

            nc.gpsimd.memset(xpad[:], 0.0)

            # input DMA chunks + weights, interleaved across the two HWDGE
            # rings so no ring is blocked behind the weight duplication.
            # Chunk rows [9,8,8,7]: the +1 conv halo means spatial half 0
            # (data rows 0..16) depends on chunks 0,1 only.
            chunks = []
            row0 = 0
            for c, nrows in enumerate((9, 8, 8, 7)):
                chunks.append((row0 * W, (row0 + nrows) * W, nrows, row0))
                row0 += nrows

            lo, hi, _, _ = chunks[0]
            nc.sync.dma_start(xin[:, lo:hi, :], x_src[:, lo:hi, :])
            lo, hi, _, _ = chunks[1]
            nc.scalar.dma_start(xin[:, lo:hi, :], x_src[:, lo:hi, :])
            # weights: 64-partition half from HBM, then duplicate onto
            # 64..127 so the K=128 contraction folds the two ic-groups
            nc.sync.dma_start(wt[0:64], w_d.rearrange("k (r t o) -> k r t o",
                                                      r=4, t=9))
            lo, hi, _, _ = chunks[2]
            nc.scalar.dma_start(xin[:, lo:hi, :], x_src[:, lo:hi, :])
            nc.sync.dma_start(wt[64:128], wt[0:64])
            lo, hi, _, _ = chunks[3]
            nc.scalar.dma_start(xin[:, lo:hi, :], x_src[:, lo:hi, :])
            nc.scalar.dma_start(bias[:], b_d)

            # ic partial-sum reduction per chunk
            for lo, hi, nrows, r0 in chunks:
                red_out = xpad[:, 1 + r0:1 + r0 + nrows, 1:33]
                with nc.allow_low_precision(reason="fp16 xs; fp32 psum accumulate"):
                    nc.vector.tensor_reduce(red_out, xin[:, lo:hi, :],
                                            axis=mybir.AxisListType.X, op=ALU.add)

            # PE warm-up: junk matmuls on the weight tile flip the HAM clock
            # gate to 8/8 before the real conv matmuls arrive (shares the psb
            # slot pool; the slot frees before the second half needs it)
            warm = psp.tile([128, 4, 512], f32, tag="psb", name="warm")
            for i in range(N_WARM):
                nc.tensor.matmul(warm[:, i % 4, :], wt[0:64, 0, 0, :],
                                 wt[0:64, i % 4, 1:5, :], start=True, stop=True)

            # conv + squash, per spatial half (16 output rows = 512 positions)
            for half in range(2):
                h0 = 16 * half
                ps = psp.tile([128, 4, 512], f32, tag="psb", name=f"ps_{half}")
                for r in range(4):
                    for t, (a, bb) in enumerate((a, bb) for a in range(3)
                                                for bb in range(3)):
                        nc.tensor.matmul(
                            ps[:, r, :],
                            wt[:, r, t, :],
                            xpad[:, h0 + a:h0 + a + 16, bb:bb + 32],
                            start=(t == 0),
                            stop=(t == 8),
                        )

                # nsq = sum_r (ps_r + bias)^2
                sq = wrk.tile([128, 4, 512], f32, tag="sq", name=f"sq_{half}")
                nc.scalar.activation(sq[:, 0:2, :], ps[:, 0:2, :],
                                     ACT.Square, bias=bias[:, :], scale=1.0)
                nc.scalar.activation(sq[:, 2:4, :], ps[:, 2:4, :],
                                     ACT.Square, bias=bias[:, :], scale=1.0)
                n01 = wrk.tile([128, 512], f32, tag="n01", name=f"n01_{half}")
                nc.gpsimd.tensor_tensor(n01, sq[:, 0, :], sq[:, 1, :], op=ALU.add)
                n23 = wrk.tile([128, 512], f32, tag="n23", name=f"n23_{half}")
                nc.gpsimd.tensor_tensor(n23, sq[:, 2, :], sq[:, 3, :], op=ALU.add)
                nsq = wrk.tile([128, 512], f32, tag="nsq", name=f"nsq_{half}")
                nc.vector.tensor_add(nsq, n01, n23)

                # scale = sqrt(nsq)/(1+nsq) = exp(0.5*ln(nsq) - ln(1+nsq)).
                # Ln+Exp share one activation-table set; Square is a filler
                # function present in every set -> no table thrash.
                lnv = wrk.tile([128, 512], f32, tag="lnv", name=f"lnv_{half}")
                nc.scalar.activation(lnv, nsq, ACT.Ln, bias=1.0, scale=1.0)
                lnn = wrk.tile([128, 512], f32, tag="lnn", name=f"lnn_{half}")
                nc.scalar.activation(lnn, nsq, ACT.Ln)
                ev = wrk.tile([128, 512], f32, tag="ev", name=f"ev_{half}")
                nc.vector.scalar_tensor_tensor(ev, lnn, 0.5, lnv,
                                               op0=ALU.mult, op1=ALU.subtract)
                sc = wrk.tile([128, 512], f32, tag="sc", name=f"sc_{half}")
                nc.scalar.activation(sc, ev, ACT.Exp)

                # out_r = (ps_r + bias) * scale, two r at a time so the first
                # output DMA overlaps the second STT
                ot = wrk.tile([128, 4, 512], f16, tag="ot", name=f"ot_{half}")
                sc_b = sc.unsqueeze(1).broadcast_to((128, 2, 512))
                eng = nc.sync if half == 0 else nc.scalar
                for rr in range(2):
                    with nc.allow_low_precision(reason="fp16 output tensor"):
                        nc.vector.scalar_tensor_tensor(
                            ot[:, 2 * rr:2 * rr + 2, :],
                            ps[:, 2 * rr:2 * rr + 2, :],
                            bias[:, :], sc_b, op0=ALU.add, op1=ALU.mult)
                    eng.dma_start(out_dst[:, half, 2 * rr:2 * rr + 2],
                                  ot[:, 2 * rr:2 * rr + 2, :])

    nc.compile()
    _CACHE["nc"] = nc
    return nc


def _prep_weights(conv_w, conv_b):
    """Host-side p4 filter transform -> lhsT tiles [c4, (r, tap, o)]."""
    w = np.asarray(conv_w, dtype=np.float32)      # (O=128, ID=16, 4, 3, 3)
    tw = np.stack(
        [np.rot90(np.roll(w, r, axis=2), k=r, axes=(3, 4)) for r in range(4)],
        axis=1,
    )                                             # (O, r, i, s, a, b)
    # lhsT[(i,s), o] per (r, tap=(a,b)):
    wh = tw.transpose(1, 4, 5, 2, 3, 0).reshape(4, 9, C4, O)   # (r, tap, c4, o)
    w_dram = np.ascontiguousarray(
        wh.transpose(2, 0, 1, 3).reshape(64, 4 * 9 * 128)
    ).astype(np.float16)
    bias16 = np.ascontiguousarray(
        (np.float32(IC) * np.asarray(conv_b, dtype=np.float32)).reshape(128, 1)
    )
    return w_dram, bias16


def make_in_maps(x, conv_w, conv_b):
    """Shard/lay out full inputs into per-core DRAM input maps."""
    x = np.asarray(x, dtype=np.float32)
    assert x.shape == (B, IC, ID, 4, H, W), x.shape
    w_dram, bias16 = _prep_weights(conv_w, conv_b)
    # (B, ic, c4, p) -> (B, u, c4, p, v) -> (B, 128, 8192), v innermost
    xr = x.reshape(B, 2, 8, C4, P).transpose(0, 1, 3, 4, 2).reshape(B, 128, P * 8)
    xr = np.ascontiguousarray(xr).astype(np.float16)
    return [
        {"xin": xr[b], "wt": w_dram, "bias16": bias16} for b in range(N_CORES)
    ]


def kernel(x, conv_w, conv_b, ln_gamma=None, ln_beta=None, k=None, ITER=None,
           **_unused):
    """Full-input, full-output entry point.  Shards batch over 8 cores."""
    from concourse.bass_utils import run_bass_kernel_spmd

    nc = _build_nc()
    in_maps = make_in_maps(x, conv_w, conv_b)
    res = run_bass_kernel_spmd(nc, in_maps, core_ids=list(range(N_CORES)))

    out = np.empty((B, OC, OD, 4, H, W), dtype=np.float32)
    for b in range(N_CORES):
        out[b] = res.results[b]["out"].astype(np.float32).reshape(
            O, 4, P).reshape(OC, OD, 4, H, W)
    return out


# revision 30
# speedup vs baseline: 1.3602x; 1.0716x over previous
"""Trainium2 Bass kernel for nn_ConvolutionalCapsules_66477503808119.

Mathematical reduction of the reference:
  * The routing chain (layernorm -> cosine sim -> top-k -> ws iterations)
    feeds only softmax(ws, axis=6) where axis 6 has size 1, so the routing
    coefficients `a` are identically 1.0 and the whole chain is dead code.
  * Therefore s_j = sum_ic preds[b, ic] and, since conv is linear,
    s_j[b] = p4conv(sum_ic x[b, ic], w, IC * bias).
  * Output = squash_over_rotation(s_j):
        nsq   = sum_r s_j^2
        scale = nsq / ((1 + nsq) * (sqrt(nsq) + 1e-8))
              ~= sqrt(nsq) / (1 + nsq) = exp(0.5*ln(nsq) - ln(1+nsq))
        out_r = scale * s_j_r

Sharding: pure data-parallel over batch, B=8 -> one batch element per core.

The kernel is HBM-bandwidth bound (all 8 cores share the chip's HBM), so
x / weights / output all travel as fp16 (PSUM accumulation stays fp32);
measured end-to-end relative error ~5.5e-3 against the fp32 reference.

Per-core device kernel:
  1. DMA x[b] as (128 part = (u2, c4=64), free = (p=1024, v8)), ic = u*8+v,
     v innermost so the ic-reduction is unit-stride and DMA runs are 4KB.
  2. VectorE reduce over v -> partial ic-sums into a zero-padded
     (128, 34, 34) spatial plane (both u-halves kept separate).
  3. Conv as 9 shifted matmuls per (rotation r, spatial half): contraction
     K = (u, c4) = 128 with weights duplicated over u (folds u for free).
  4. Squash: Square/Ln/Exp on ScalarE (one activation-table set),
     adds on GpSimd, scale + out on VectorE.
  5. DMA out (128 part = o, free = (r, p)) -> (OC, OD, 4, H, W) per batch.
"""

import numpy as np

B, IC, ID, OC, OD = 8, 16, 16, 8, 16
KS, PAD, H, W = 3, 1, 32, 32
C4 = ID * 4          # 64 conv input channels
P = H * W            # 1024 spatial positions
O = OC * OD          # 128 output channels (pre-rotation)
N_CORES = 8
N_WARM = 16          # junk matmuls that warm the PE HAM clock gate

_CACHE = {}


def _install_act_tables():
    """Point the BIR compiler at a reordered act_info.json so Ln/Exp/Square
    all resolve to the one table set that contains them all
    (natural_log_exp_and_others) -> zero mid-kernel table reloads."""
    import glob
    import json
    import os

    if os.environ.get("BASS_ACT_ROOT_JSON_PATH"):
        return
    try:
        import neuronxcc
        base = os.path.join(os.path.dirname(neuronxcc.__file__),
                            "pwp", "pwp_bin_trainium")
        info = json.load(open(os.path.join(base, "act_info.json")))
        sets = info["act_func_sets"]
        sets.sort(key=lambda s: s["name"] != "natural_log_exp_and_others")
        dst = "/tmp/capsules_act_tables"
        os.makedirs(dst, exist_ok=True)
        for f in glob.glob(os.path.join(base, "*")):
            link = os.path.join(dst, os.path.basename(f))
            if not os.path.exists(link):
                os.symlink(f, link)
        os.unlink(os.path.join(dst, "act_info.json"))
        with open(os.path.join(dst, "act_info.json"), "w") as fh:
            json.dump(info, fh)
        os.environ["BASS_ACT_ROOT_JSON_PATH"] = os.path.join(dst,
                                                             "act_info.json")
    except Exception:
        pass  # fall back to the stock tables (correct, just slower)


def _build_nc():
    """Build + compile the single-core Bass program (shared by all 8 cores)."""
    if "nc" in _CACHE:
        return _CACHE["nc"]

    import concourse.mybir as mybir
    import concourse.tile as tile
    from concourse import bacc

    f32 = mybir.dt.float32
    f16 = mybir.dt.float16
    ACT = mybir.ActivationFunctionType
    ALU = mybir.AluOpType

    nc = bacc.Bacc("TRN2", target_bir_lowering=False, debug=False)

    x_d = nc.dram_tensor("xin", (128, P * 8), f16, kind="ExternalInput").ap()
    w_d = nc.dram_tensor("wt", (64, 4 * 9 * 128), f16, kind="ExternalInput").ap()
    b_d = nc.dram_tensor("bias16", (128, 1), f32, kind="ExternalInput").ap()
    out_d = nc.dram_tensor("out", (128, 4 * P), f16, kind="ExternalOutput").ap()

    x_src = x_d.rearrange("q (p v) -> q p v", v=8)
    out_dst = out_d.rearrange("o (r hh p) -> o hh r p", r=4, hh=2)

    with tile.TileContext(nc) as tc:
        with tc.tile_pool(name="cst", bufs=1) as cst, \
             tc.tile_pool(name="wrk", bufs=2) as wrk, \
             tc.tile_pool(name="ps", bufs=2, space="PSUM") as psp:

            xin = cst.tile([128, P, 8], f16)
            xpad = cst.tile([128, 34, 34], f16)
            wt = cst.tile([128, 4, 9, 128], f16)
            bias = cst.tile([128, 1], f32)

            nc.gpsimd.memset(xpad[:], 0.0)

            # input DMA chunks + weights, interleaved across the two HWDGE
            # rings so no ring is blocked behind the weight duplication.
            # Chunk rows [9,8,8,7]: the +1 conv halo means spatial half 0
            # (data rows 0..16) depends on chunks 0,1 only.
            chunks = []
            row0 = 0
            for c, nrows in enumerate((9, 8, 8, 7)):
                chunks.append((row0 * W, (row0 + nrows) * W, nrows, row0))
                row0 += nrows

            # weights: 64-partition half from HBM first on the sync ring (the
            # warm-up matmuls need it); duplication onto 64..127 (so the
            # K=128 contraction folds the two ic-groups) rides the gpsimd
            # SWDGE ring -- only the real conv matmuls need it.
            nc.sync.dma_start(wt[0:64], w_d.rearrange("k (r t o) -> k r t o",
                                                      r=4, t=9))
            nc.scalar.dma_start(bias[:], b_d)
            for c, (lo, hi, _, _) in enumerate(chunks):
                eng = nc.sync if c % 2 == 0 else nc.scalar
                eng.dma_start(xin[:, lo:hi, :], x_src[:, lo:hi, :])
            nc.gpsimd.dma_start(wt[64:128], wt[0:64])

            # ic partial-sum reduction per chunk
            for lo, hi, nrows, r0 in chunks:
                red_out = xpad[:, 1 + r0:1 + r0 + nrows, 1:33]
                with nc.allow_low_precision(reason="fp16 xs; fp32 psum accumulate"):
                    nc.vector.tensor_reduce(red_out, xin[:, lo:hi, :],
                                            axis=mybir.AxisListType.X, op=ALU.add)

            # PE warm-up: junk matmuls on the weight tile flip the HAM clock
            # gate to 8/8 before the real conv matmuls arrive (shares the psb
            # slot pool; the slot frees before the second half needs it)
            warm = psp.tile([128, 4, 512], f32, tag="psb", name="warm")
            for i in range(N_WARM):
                nc.tensor.matmul(warm[:, i % 4, :], wt[0:64, 0, 0, :],
                                 wt[0:64, i % 4, 1:5, :], start=True, stop=True)

            # conv + squash, per spatial half (16 output rows = 512 positions)
            for half in range(2):
                h0 = 16 * half
                ps = psp.tile([128, 4, 512], f32, tag="psb", name=f"ps_{half}")
                for r in range(4):
                    for t, (a, bb) in enumerate((a, bb) for a in range(3)
                                                for bb in range(3)):
                        nc.tensor.matmul(
                            ps[:, r, :],
                            wt[:, r, t, :],
                            xpad[:, h0 + a:h0 + a + 16, bb:bb + 32],
                            start=(t == 0),
                            stop=(t == 8),
                        )

                # nsq = sum_r (ps_r + bias)^2
                sq = wrk.tile([128, 4, 512], f32, tag="sq", name=f"sq_{half}")
                nc.scalar.activation(sq[:, 0:2, :], ps[:, 0:2, :],
                                     ACT.Square, bias=bias[:, :], scale=1.0)
                nc.scalar.activation(sq[:, 2:4, :], ps[:, 2:4, :],
                                     ACT.Square, bias=bias[:, :], scale=1.0)
                n01 = wrk.tile([128, 512], f32, tag="n01", name=f"n01_{half}")
                nc.vector.tensor_add(n01, sq[:, 0, :], sq[:, 1, :])
                n23 = wrk.tile([128, 512], f32, tag="n23", name=f"n23_{half}")
                nc.vector.tensor_add(n23, sq[:, 2, :], sq[:, 3, :])
                nsq = wrk.tile([128, 512], f32, tag="nsq", name=f"nsq_{half}")
                nc.vector.tensor_add(nsq, n01, n23)

                # scale = sqrt(nsq)/(1+nsq) = exp(0.5*ln(nsq) - ln(1+nsq)).
                # Ln+Exp share one activation-table set; Square is a filler
                # function present in every set -> no table thrash.
                lnv = wrk.tile([128, 512], f32, tag="lnv", name=f"lnv_{half}")
                nc.scalar.activation(lnv, nsq, ACT.Ln, bias=1.0, scale=1.0)
                lnn = wrk.tile([128, 512], f32, tag="lnn", name=f"lnn_{half}")
                nc.scalar.activation(lnn, nsq, ACT.Ln)
                ev = wrk.tile([128, 512], f32, tag="ev", name=f"ev_{half}")
                nc.vector.scalar_tensor_tensor(ev, lnn, 0.5, lnv,
                                               op0=ALU.mult, op1=ALU.subtract)
                sc = wrk.tile([128, 512], f32, tag="sc", name=f"sc_{half}")
                nc.scalar.activation(sc, ev, ACT.Exp)

                # out_r = (ps_r + bias) * scale, two r at a time so the first
                # output DMA overlaps the second STT
                ot = wrk.tile([128, 4, 512], f16, tag="ot", name=f"ot_{half}")
                sc_b = sc.unsqueeze(1).broadcast_to((128, 2, 512))
                eng = nc.sync if half == 0 else nc.scalar
                for rr in range(2):
                    with nc.allow_low_precision(reason="fp16 output tensor"):
                        nc.vector.scalar_tensor_tensor(
                            ot[:, 2 * rr:2 * rr + 2, :],
                            ps[:, 2 * rr:2 * rr + 2, :],
                            bias[:, :], sc_b, op0=ALU.add, op1=ALU.mult)
                    eng.dma_start(out_dst[:, half, 2 * rr:2 * rr + 2],
                                  ot[:, 2 * rr:2 * rr + 2, :])

    nc.compile()
    _CACHE["nc"] = nc
    return nc


def _prep_weights(conv_w, conv_b):
    """Host-side p4 filter transform -> lhsT tiles [c4, (r, tap, o)]."""
    w = np.asarray(conv_w, dtype=np.float32)      # (O=128, ID=16, 4, 3, 3)
    tw = np.stack(
        [np.rot90(np.roll(w, r, axis=2), k=r, axes=(3, 4)) for r in range(4)],
        axis=1,
    )                                             # (O, r, i, s, a, b)
    # lhsT[(i,s), o] per (r, tap=(a,b)):
    wh = tw.transpose(1, 4, 5, 2, 3, 0).reshape(4, 9, C4, O)   # (r, tap, c4, o)
    w_dram = np.ascontiguousarray(
        wh.transpose(2, 0, 1, 3).reshape(64, 4 * 9 * 128)
    ).astype(np.float16)
    bias16 = np.ascontiguousarray(
        (np.float32(IC) * np.asarray(conv_b, dtype=np.float32)).reshape(128, 1)
    )
    return w_dram, bias16


def make_in_maps(x, conv_w, conv_b):
    """Shard/lay out full inputs into per-core DRAM input maps."""
    x = np.asarray(x, dtype=np.float32)
    assert x.shape == (B, IC, ID, 4, H, W), x.shape
    w_dram, bias16 = _prep_weights(conv_w, conv_b)
    # (B, ic, c4, p) -> (B, u, c4, p, v) -> (B, 128, 8192), v innermost
    xr = x.reshape(B, 2, 8, C4, P).transpose(0, 1, 3, 4, 2).reshape(B, 128, P * 8)
    xr = np.ascontiguousarray(xr).astype(np.float16)
    return [
        {"xin": xr[b], "wt": w_dram, "bias16": bias16} for b in range(N_CORES)
    ]


def kernel(x, conv_w, conv_b, ln_gamma=None, ln_beta=None, k=None, ITER=None,
           **_unused):
    """Full-input, full-output entry point.  Shards batch over 8 cores."""
    from concourse.bass_utils import run_bass_kernel_spmd

    nc = _build_nc()
    in_maps = make_in_maps(x, conv_w, conv_b)
    res = run_bass_kernel_spmd(nc, in_maps, core_ids=list(range(N_CORES)))

    out = np.empty((B, OC, OD, 4, H, W), dtype=np.float32)
    for b in range(N_CORES):
        out[b] = res.results[b]["out"].astype(np.float32).reshape(
            O, 4, P).reshape(OC, OD, 4, H, W)
    return out


# revision 31
# speedup vs baseline: 1.3717x; 1.0085x over previous
"""Trainium2 Bass kernel for nn_ConvolutionalCapsules_66477503808119.

Mathematical reduction of the reference:
  * The routing chain (layernorm -> cosine sim -> top-k -> ws iterations)
    feeds only softmax(ws, axis=6) where axis 6 has size 1, so the routing
    coefficients `a` are identically 1.0 and the whole chain is dead code.
  * Therefore s_j = sum_ic preds[b, ic] and, since conv is linear,
    s_j[b] = p4conv(sum_ic x[b, ic], w, IC * bias).
  * Output = squash_over_rotation(s_j):
        nsq   = sum_r s_j^2
        scale = nsq / ((1 + nsq) * (sqrt(nsq) + 1e-8))
              ~= sqrt(nsq) / (1 + nsq) = exp(0.5*ln(nsq) - ln(1+nsq))
        out_r = scale * s_j_r

Sharding: pure data-parallel over batch, B=8 -> one batch element per core.

The kernel is HBM-bandwidth bound (all 8 cores share the chip's HBM), so
x / weights / output all travel as fp16 (PSUM accumulation stays fp32);
measured end-to-end relative error ~5.5e-3 against the fp32 reference.

Per-core device kernel:
  1. DMA x[b] as (128 part = (u2, c4=64), free = (p=1024, v8)), ic = u*8+v,
     v innermost so the ic-reduction is unit-stride and DMA runs are 4KB.
  2. VectorE reduce over v -> partial ic-sums into a zero-padded
     (128, 34, 34) spatial plane (both u-halves kept separate).
  3. Conv as 9 shifted matmuls per (rotation r, spatial half): contraction
     K = (u, c4) = 128 with weights duplicated over u (folds u for free).
  4. Squash: Square/Ln/Exp on ScalarE (one activation-table set),
     adds on GpSimd, scale + out on VectorE.
  5. DMA out (128 part = o, free = (r, p)) -> (OC, OD, 4, H, W) per batch.
"""

import numpy as np

B, IC, ID, OC, OD = 8, 16, 16, 8, 16
KS, PAD, H, W = 3, 1, 32, 32
C4 = ID * 4          # 64 conv input channels
P = H * W            # 1024 spatial positions
O = OC * OD          # 128 output channels (pre-rotation)
N_CORES = 8
N_WARM = 16          # junk matmuls that warm the PE HAM clock gate

_CACHE = {}


def _build_nc():
    """Build + compile the single-core Bass program (shared by all 8 cores)."""
    if "nc" in _CACHE:
        return _CACHE["nc"]

    import concourse.mybir as mybir
    import concourse.tile as tile
    from concourse import bacc

    f32 = mybir.dt.float32
    f16 = mybir.dt.float16
    ACT = mybir.ActivationFunctionType
    ALU = mybir.AluOpType

    nc = bacc.Bacc("TRN2", target_bir_lowering=False, debug=False)

    x_d = nc.dram_tensor("xin", (128, P * 8), f16, kind="ExternalInput").ap()
    w_d = nc.dram_tensor("wt", (64, 4 * 9 * 128), f16, kind="ExternalInput").ap()
    b_d = nc.dram_tensor("bias16", (128, 1), f32, kind="ExternalInput").ap()
    out_d = nc.dram_tensor("out", (128, 4 * P), f16, kind="ExternalOutput").ap()

    x_src = x_d.rearrange("q (p v) -> q p v", v=8)
    out_dst = out_d.rearrange("o (r hh p) -> o hh r p", r=4, hh=2)

    with tile.TileContext(nc) as tc:
        with tc.tile_pool(name="cst", bufs=1) as cst, \
             tc.tile_pool(name="wrk", bufs=2) as wrk, \
             tc.tile_pool(name="ps", bufs=2, space="PSUM") as psp:

            xin = cst.tile([128, P, 8], f16)
            xpad = cst.tile([128, 34, 34], f16)
            wt = cst.tile([128, 4, 9, 128], f16)
            bias = cst.tile([128, 1], f32)

            nc.gpsimd.memset(xpad[:], 0.0)

            # input DMA chunks + weights, interleaved across the two HWDGE
            # rings so no ring is blocked behind the weight duplication.
            # Chunk rows [9,8,8,7]: the +1 conv halo means spatial half 0
            # (data rows 0..16) depends on chunks 0,1 only.
            chunks = []
            row0 = 0
            for c, nrows in enumerate((9, 8, 8, 7)):
                chunks.append((row0 * W, (row0 + nrows) * W, nrows, row0))
                row0 += nrows

            # weights: 64-partition half from HBM first on the sync ring (the
            # warm-up matmuls need it); duplication onto 64..127 (so the
            # K=128 contraction folds the two ic-groups) rides the gpsimd
            # SWDGE ring -- only the real conv matmuls need it.
            nc.sync.dma_start(wt[0:64], w_d.rearrange("k (r t o) -> k r t o",
                                                      r=4, t=9))
            nc.scalar.dma_start(bias[:], b_d)
            for c, (lo, hi, _, _) in enumerate(chunks):
                eng = nc.sync if c % 2 == 0 else nc.scalar
                eng.dma_start(xin[:, lo:hi, :], x_src[:, lo:hi, :])
            nc.gpsimd.dma_start(wt[64:128], wt[0:64])

            # ic partial-sum reduction per chunk
            for lo, hi, nrows, r0 in chunks:
                red_out = xpad[:, 1 + r0:1 + r0 + nrows, 1:33]
                with nc.allow_low_precision(reason="fp16 xs; fp32 psum accumulate"):
                    nc.vector.tensor_reduce(red_out, xin[:, lo:hi, :],
                                            axis=mybir.AxisListType.X, op=ALU.add)

            # PE warm-up: junk matmuls on the weight tile flip the HAM clock
            # gate to 8/8 before the real conv matmuls arrive (shares the psb
            # slot pool; the slot frees before the second half needs it)
            warm = psp.tile([128, 4, 512], f32, tag="psb", name="warm")
            for i in range(N_WARM):
                nc.tensor.matmul(warm[:, i % 4, :], wt[0:64, 0, 0, :],
                                 wt[0:64, i % 4, 1:5, :], start=True, stop=True)

            # conv + squash, per spatial half (16 output rows = 512 positions)
            for half in range(2):
                h0 = 16 * half
                ps = psp.tile([128, 4, 512], f32, tag="psb", name=f"ps_{half}")
                for r in range(4):
                    for t, (a, bb) in enumerate((a, bb) for a in range(3)
                                                for bb in range(3)):
                        nc.tensor.matmul(
                            ps[:, r, :],
                            wt[:, r, t, :],
                            xpad[:, h0 + a:h0 + a + 16, bb:bb + 32],
                            start=(t == 0),
                            stop=(t == 8),
                        )

                # nsq = sum_r (ps_r + bias)^2
                sq = wrk.tile([128, 4, 512], f32, tag="sq", name=f"sq_{half}")
                nc.scalar.activation(sq[:, 0:2, :], ps[:, 0:2, :],
                                     ACT.Square, bias=bias[:, :], scale=1.0)
                nc.scalar.activation(sq[:, 2:4, :], ps[:, 2:4, :],
                                     ACT.Square, bias=bias[:, :], scale=1.0)
                n01 = wrk.tile([128, 512], f32, tag="n01", name=f"n01_{half}")
                nc.vector.tensor_add(n01, sq[:, 0, :], sq[:, 1, :])
                n23 = wrk.tile([128, 512], f32, tag="n23", name=f"n23_{half}")
                nc.vector.tensor_add(n23, sq[:, 2, :], sq[:, 3, :])
                nsq = wrk.tile([128, 512], f32, tag="nsq", name=f"nsq_{half}")
                nc.vector.tensor_add(nsq, n01, n23)

                # scale = sqrt(nsq)/(1+nsq) = exp(0.5*ln(nsq) - ln(1+nsq)).
                # Ln+Exp share one activation-table set; Square is a filler
                # function present in every set -> no table thrash.
                lnv = wrk.tile([128, 512], f32, tag="lnv", name=f"lnv_{half}")
                nc.scalar.activation(lnv, nsq, ACT.Ln, bias=1.0, scale=1.0)
                lnn = wrk.tile([128, 512], f32, tag="lnn", name=f"lnn_{half}")
                nc.scalar.activation(lnn, nsq, ACT.Ln)
                ev = wrk.tile([128, 512], f32, tag="ev", name=f"ev_{half}")
                nc.vector.scalar_tensor_tensor(ev, lnn, 0.5, lnv,
                                               op0=ALU.mult, op1=ALU.subtract)
                sc = wrk.tile([128, 512], f32, tag="sc", name=f"sc_{half}")
                nc.scalar.activation(sc, ev, ACT.Exp)

                # out_r = (ps_r + bias) * scale, two r at a time so the first
                # output DMA overlaps the second STT
                ot = wrk.tile([128, 4, 512], f16, tag="ot", name=f"ot_{half}")
                sc_b = sc.unsqueeze(1).broadcast_to((128, 2, 512))
                eng = nc.sync if half == 0 else nc.scalar
                for rr in range(2):
                    with nc.allow_low_precision(reason="fp16 output tensor"):
                        nc.vector.scalar_tensor_tensor(
                            ot[:, 2 * rr:2 * rr + 2, :],
                            ps[:, 2 * rr:2 * rr + 2, :],
                            bias[:, :], sc_b, op0=ALU.add, op1=ALU.mult)
                    eng.dma_start(out_dst[:, half, 2 * rr:2 * rr + 2],
                                  ot[:, 2 * rr:2 * rr + 2, :])

    nc.compile()
    _CACHE["nc"] = nc
    return nc


def _prep_weights(conv_w, conv_b):
    """Host-side p4 filter transform -> lhsT tiles [c4, (r, tap, o)]."""
    w = np.asarray(conv_w, dtype=np.float32)      # (O=128, ID=16, 4, 3, 3)
    tw = np.stack(
        [np.rot90(np.roll(w, r, axis=2), k=r, axes=(3, 4)) for r in range(4)],
        axis=1,
    )                                             # (O, r, i, s, a, b)
    # lhsT[(i,s), o] per (r, tap=(a,b)):
    wh = tw.transpose(1, 4, 5, 2, 3, 0).reshape(4, 9, C4, O)   # (r, tap, c4, o)
    w_dram = np.ascontiguousarray(
        wh.transpose(2, 0, 1, 3).reshape(64, 4 * 9 * 128)
    ).astype(np.float16)
    bias16 = np.ascontiguousarray(
        (np.float32(IC) * np.asarray(conv_b, dtype=np.float32)).reshape(128, 1)
    )
    return w_dram, bias16


def make_in_maps(x, conv_w, conv_b):
    """Shard/lay out full inputs into per-core DRAM input maps."""
    x = np.asarray(x, dtype=np.float32)
    assert x.shape == (B, IC, ID, 4, H, W), x.shape
    w_dram, bias16 = _prep_weights(conv_w, conv_b)
    # (B, ic, c4, p) -> (B, u, c4, p, v) -> (B, 128, 8192), v innermost
    xr = x.reshape(B, 2, 8, C4, P).transpose(0, 1, 3, 4, 2).reshape(B, 128, P * 8)
    xr = np.ascontiguousarray(xr).astype(np.float16)
    return [
        {"xin": xr[b], "wt": w_dram, "bias16": bias16} for b in range(N_CORES)
    ]


def kernel(x, conv_w, conv_b, ln_gamma=None, ln_beta=None, k=None, ITER=None,
           **_unused):
    """Full-input, full-output entry point.  Shards batch over 8 cores."""
    from concourse.bass_utils import run_bass_kernel_spmd

    nc = _build_nc()
    in_maps = make_in_maps(x, conv_w, conv_b)
    res = run_bass_kernel_spmd(nc, in_maps, core_ids=list(range(N_CORES)))

    out = np.empty((B, OC, OD, 4, H, W), dtype=np.float32)
    for b in range(N_CORES):
        out[b] = res.results[b]["out"].astype(np.float32).reshape(
            O, 4, P).reshape(OC, OD, 4, H, W)
    return out


# revision 33
# speedup vs baseline: 1.4194x; 1.0347x over previous
"""Trainium2 Bass kernel for nn_ConvolutionalCapsules_66477503808119.

Mathematical reduction of the reference:
  * The routing chain (layernorm -> cosine sim -> top-k -> ws iterations)
    feeds only softmax(ws, axis=6) where axis 6 has size 1, so the routing
    coefficients `a` are identically 1.0 and the whole chain is dead code.
  * Therefore s_j = sum_ic preds[b, ic] and, since conv is linear,
    s_j[b] = p4conv(sum_ic x[b, ic], w, IC * bias).
  * Output = squash_over_rotation(s_j):
        nsq   = sum_r s_j^2
        scale = nsq / ((1 + nsq) * (sqrt(nsq) + 1e-8))
              ~= sqrt(nsq) / (1 + nsq) = exp(0.5*ln(nsq) - ln(1+nsq))
        out_r = scale * s_j_r

Sharding: pure data-parallel over batch, B=8 -> one batch element per core.

The kernel is HBM-bandwidth bound (all 8 cores share the chip's HBM), so
x / weights / output all travel as fp16 (PSUM accumulation stays fp32);
measured end-to-end relative error ~5.5e-3 against the fp32 reference.

Per-core device kernel:
  1. DMA x[b] as (128 part = (u2, c4=64), free = (p=1024, v8)), ic = u*8+v,
     v innermost so the ic-reduction is unit-stride and DMA runs are 4KB.
  2. VectorE reduce over v -> partial ic-sums into a zero-padded
     (128, 34, 34) spatial plane (both u-halves kept separate).
  3. Conv as 9 shifted matmuls per (rotation r, spatial half): contraction
     K = (u, c4) = 128 with weights duplicated over u (folds u for free).
  4. Squash: Square/Ln/Exp on ScalarE, adds + scale + out on VectorE.
  5. DMA out (128 part = o, free = (r, p)) -> (OC, OD, 4, H, W) per batch.
"""

import numpy as np

B, IC, ID, OC, OD = 8, 16, 16, 8, 16
KS, PAD, H, W = 3, 1, 32, 32
C4 = ID * 4          # 64 conv input channels
P = H * W            # 1024 spatial positions
O = OC * OD          # 128 output channels (pre-rotation)
N_CORES = 8
N_WARM = 16          # junk matmuls that warm the PE HAM clock gate

_CACHE = {}


def _build_nc():
    """Build + compile the single-core Bass program (shared by all 8 cores)."""
    if "nc" in _CACHE:
        return _CACHE["nc"]

    import concourse.mybir as mybir
    import concourse.tile as tile
    from concourse import bacc

    f32 = mybir.dt.float32
    f16 = mybir.dt.float16
    ACT = mybir.ActivationFunctionType
    ALU = mybir.AluOpType

    nc = bacc.Bacc("TRN2", target_bir_lowering=False, debug=False)

    x_d = nc.dram_tensor("xin", (128, P * 8), f16, kind="ExternalInput").ap()
    w_d = nc.dram_tensor("wt", (64, 4 * 9 * 128), f16, kind="ExternalInput").ap()
    b_d = nc.dram_tensor("bias16", (128, 1), f32, kind="ExternalInput").ap()
    out_d = nc.dram_tensor("out", (128, 4 * P), f16, kind="ExternalOutput").ap()

    x_src = x_d.rearrange("q (p v) -> q p v", v=8)
    out_dst = out_d.rearrange("o (r hh p) -> o hh r p", r=4, hh=2)

    with tile.TileContext(nc) as tc:
        with tc.tile_pool(name="cst", bufs=1) as cst, \
             tc.tile_pool(name="wrk", bufs=2) as wrk, \
             tc.tile_pool(name="ps", bufs=2, space="PSUM") as psp:

            xin = cst.tile([128, P, 8], f16)
            xpad = cst.tile([128, 34, 34], f16)
            wt = cst.tile([128, 4, 9, 128], f16)
            bias = cst.tile([128, 1], f32)

            nc.gpsimd.memset(xpad[:], 0.0)

            # input DMA chunks + weights, interleaved across the two HWDGE
            # rings so no ring is blocked behind the weight duplication.
            # Chunk rows [5,4,4,4,4,4,4,3]: with the +1 conv halo, spatial
            # half 0 (data rows 0..16) depends on chunks 0..3 only, and the
            # finer grain starts the reduce->matmul pipeline earlier.
            chunks = []
            row0 = 0
            for c, nrows in enumerate((5, 4, 4, 4, 4, 4, 4, 3)):
                chunks.append((row0 * W, (row0 + nrows) * W, nrows, row0))
                row0 += nrows

            # first x chunk beats the weights onto the sync ring; the weight
            # duplication onto partitions 64..127 (so the K=128 contraction
            # folds the two ic-groups) rides the gpsimd SWDGE ring -- only
            # the real conv matmuls need it, warm-up uses wt[0:64].
            lo, hi, _, _ = chunks[0]
            nc.sync.dma_start(xin[:, lo:hi, :], x_src[:, lo:hi, :])
            nc.scalar.dma_start(bias[:], b_d)
            lo, hi, _, _ = chunks[1]
            nc.scalar.dma_start(xin[:, lo:hi, :], x_src[:, lo:hi, :])
            nc.sync.dma_start(wt[0:64], w_d.rearrange("k (r t o) -> k r t o",
                                                      r=4, t=9))
            for c, (lo, hi, _, _) in enumerate(chunks[2:], start=2):
                eng = nc.sync if c % 2 == 0 else nc.scalar
                eng.dma_start(xin[:, lo:hi, :], x_src[:, lo:hi, :])
            nc.gpsimd.dma_start(wt[64:128], wt[0:64])

            # ic partial-sum reduction per chunk
            for lo, hi, nrows, r0 in chunks:
                red_out = xpad[:, 1 + r0:1 + r0 + nrows, 1:33]
                with nc.allow_low_precision(reason="fp16 xs; fp32 psum accumulate"):
                    nc.vector.tensor_reduce(red_out, xin[:, lo:hi, :],
                                            axis=mybir.AxisListType.X, op=ALU.add)

            # PE warm-up: junk matmuls on the weight tile flip the HAM clock
            # gate to 8/8 before the real conv matmuls arrive (shares the psb
            # slot pool; the slot frees before the second half needs it)
            warm = psp.tile([128, 4, 512], f32, tag="psb", name="warm")
            for i in range(N_WARM):
                nc.tensor.matmul(warm[:, i % 4, :], wt[0:64, 0, 0, :],
                                 wt[0:64, i % 4, 1:5, :], start=True, stop=True)

            # conv + squash, per spatial half (16 output rows = 512 positions)
            for half in range(2):
                h0 = 16 * half
                ps = psp.tile([128, 4, 512], f32, tag="psb", name=f"ps_{half}")
                for r in range(4):
                    for t, (a, bb) in enumerate((a, bb) for a in range(3)
                                                for bb in range(3)):
                        nc.tensor.matmul(
                            ps[:, r, :],
                            wt[:, r, t, :],
                            xpad[:, h0 + a:h0 + a + 16, bb:bb + 32],
                            start=(t == 0),
                            stop=(t == 8),
                        )

                # nsq = sum_r (ps_r + bias)^2
                sq = wrk.tile([128, 4, 512], f32, tag="sq", name=f"sq_{half}")
                nc.scalar.activation(sq[:, 0:2, :], ps[:, 0:2, :],
                                     ACT.Square, bias=bias[:, :], scale=1.0)
                nc.scalar.activation(sq[:, 2:4, :], ps[:, 2:4, :],
                                     ACT.Square, bias=bias[:, :], scale=1.0)
                n01 = wrk.tile([128, 512], f32, tag="n01", name=f"n01_{half}")
                nc.vector.tensor_add(n01, sq[:, 0, :], sq[:, 1, :])
                n23 = wrk.tile([128, 512], f32, tag="n23", name=f"n23_{half}")
                nc.vector.tensor_add(n23, sq[:, 2, :], sq[:, 3, :])
                nsq = wrk.tile([128, 512], f32, tag="nsq", name=f"nsq_{half}")
                nc.vector.tensor_add(nsq, n01, n23)

                # scale = sqrt(nsq)/(1+nsq) = exp(0.5*ln(nsq) - ln(1+nsq)).
                # Ln+Exp share one activation-table set; Square is a filler
                # function present in every set -> no table thrash.
                lnv = wrk.tile([128, 512], f32, tag="lnv", name=f"lnv_{half}")
                nc.scalar.activation(lnv, nsq, ACT.Ln, bias=1.0, scale=1.0)
                lnn = wrk.tile([128, 512], f32, tag="lnn", name=f"lnn_{half}")
                nc.scalar.activation(lnn, nsq, ACT.Ln)
                ev = wrk.tile([128, 512], f32, tag="ev", name=f"ev_{half}")
                nc.vector.scalar_tensor_tensor(ev, lnn, 0.5, lnv,
                                               op0=ALU.mult, op1=ALU.subtract)
                sc = wrk.tile([128, 512], f32, tag="sc", name=f"sc_{half}")
                nc.scalar.activation(sc, ev, ACT.Exp)

                # out_r = (ps_r + bias) * scale, two r at a time so the first
                # output DMA overlaps the second STT
                ot = wrk.tile([128, 4, 512], f16, tag="ot", name=f"ot_{half}")
                sc_b = sc.unsqueeze(1).broadcast_to((128, 2, 512))
                eng = nc.sync if half == 0 else nc.scalar
                for rr in range(2):
                    with nc.allow_low_precision(reason="fp16 output tensor"):
                        nc.vector.scalar_tensor_tensor(
                            ot[:, 2 * rr:2 * rr + 2, :],
                            ps[:, 2 * rr:2 * rr + 2, :],
                            bias[:, :], sc_b, op0=ALU.add, op1=ALU.mult)
                    eng.dma_start(out_dst[:, half, 2 * rr:2 * rr + 2],
                                  ot[:, 2 * rr:2 * rr + 2, :])

    nc.compile()
    _CACHE["nc"] = nc
    return nc


def _prep_weights(conv_w, conv_b):
    """Host-side p4 filter transform -> lhsT tiles [c4, (r, tap, o)]."""
    w = np.asarray(conv_w, dtype=np.float32)      # (O=128, ID=16, 4, 3, 3)
    tw = np.stack(
        [np.rot90(np.roll(w, r, axis=2), k=r, axes=(3, 4)) for r in range(4)],
        axis=1,
    )                                             # (O, r, i, s, a, b)
    # lhsT[(i,s), o] per (r, tap=(a,b)):
    wh = tw.transpose(1, 4, 5, 2, 3, 0).reshape(4, 9, C4, O)   # (r, tap, c4, o)
    w_dram = np.ascontiguousarray(
        wh.transpose(2, 0, 1, 3).reshape(64, 4 * 9 * 128)
    ).astype(np.float16)
    bias16 = np.ascontiguousarray(
        (np.float32(IC) * np.asarray(conv_b, dtype=np.float32)).reshape(128, 1)
    )
    return w_dram, bias16


def make_in_maps(x, conv_w, conv_b):
    """Shard/lay out full inputs into per-core DRAM input maps."""
    x = np.asarray(x, dtype=np.float32)
    assert x.shape == (B, IC, ID, 4, H, W), x.shape
    w_dram, bias16 = _prep_weights(conv_w, conv_b)
    # (B, ic, c4, p) -> (B, u, c4, p, v) -> (B, 128, 8192), v innermost
    xr = x.reshape(B, 2, 8, C4, P).transpose(0, 1, 3, 4, 2).reshape(B, 128, P * 8)
    xr = np.ascontiguousarray(xr).astype(np.float16)
    return [
        {"xin": xr[b], "wt": w_dram, "bias16": bias16} for b in range(N_CORES)
    ]


def kernel(x, conv_w, conv_b, ln_gamma=None, ln_beta=None, k=None, ITER=None,
           **_unused):
    """Full-input, full-output entry point.  Shards batch over 8 cores."""
    from concourse.bass_utils import run_bass_kernel_spmd

    nc = _build_nc()
    in_maps = make_in_maps(x, conv_w, conv_b)
    res = run_bass_kernel_spmd(nc, in_maps, core_ids=list(range(N_CORES)))

    out = np.empty((B, OC, OD, 4, H, W), dtype=np.float32)
    for b in range(N_CORES):
        out[b] = res.results[b]["out"].astype(np.float32).reshape(
            O, 4, P).reshape(OC, OD, 4, H, W)
    return out
